# revision 17
# baseline (speedup 1.0000x reference)
"""Trainium2 Bass kernel for ClassFeatureMemoryBank proto-contrastive loss (v7).

loss = mean_r [ logsumexp_c(f_hat_r . p_c / T) - (f_hat_r . p_label_r / T) ]

v7 strategy (vs v2 baseline): eliminate ALL per-tile ACT/DVE instructions.
  - Host: sort rows by label, pad each class to 128-row tiles -> every tile
    is single-label. Ship features ROW-major (fR), tile-blocked.
  - ACT (batched): f2 = Square(fR); expb = Exp(logits) with constant scale.
  - GPSIMD: first tree level of both reductions (sum over d; sum over c).
  - DVE (batched): rest of q-tree, Newton rsqrt (s=1/(T*||f||)) on [128,nt]
    layout, f_hat = fR * s (free-dim broadcast), sumexp tree.
  - DMA xbar: blocked transpose f_hatR -> f_hatT per 18-tile chunk.
  - PE: per tile ONE stationary f_hatT, two matmuls: logits (protosT,
    padded to 160 cols with zeros -> sumexp gets +10, host subtracts) and
    picked (gT2 = per-tile label prototype column, FD=1).
  - Host: loss = mean(log(sumexp-10) - picked) over valid rows.
"""
import sys

sys.path.insert(0, "/opt/trn_rl_repo")

import numpy as np
import ml_dtypes
from contextlib import ExitStack

import concourse.bass as bass
import concourse.tile as tile
from concourse import bacc, mybir
from concourse.bass_utils import run_bass_kernel_spmd

F32 = mybir.dt.float32
BF16 = mybir.dt.bfloat16
I32 = mybir.dt.int32
ALU = mybir.AluOpType
AX = mybir.AxisListType

N_CORES = 8
D = 128
C = 150
CP = 160              # padded class columns (150 real + 10 zero)
SLOT = 170            # psum slot stride (3 slots * 170 * 4B = 2040 <= bank)
TEMP = 0.15
CH_T = 18             # tiles per chunk
EB = 9                # tiles per exp batch (2 per chunk)
NEWT_CH = 4           # chunks per newton batch

N_FULL = 1_000_000

_NC_CACHE = {}


def build_nc(nt: int):
    """nt tiles of 128 rows per core; nt % CH_T == 0."""
    assert nt % CH_T == 0
    nch = nt // CH_T
    half = (nt + 1) // 2
    assert half <= 512

    nc = bacc.Bacc("TRN2", target_bir_lowering=False, debug=False)
    fR = nc.dram_tensor("fR", [nch, 128, CH_T, D], BF16,
                        kind="ExternalInput").ap()
    gT2 = nc.dram_tensor("gT2", [128, nt], BF16, kind="ExternalInput").ap()
    protosT = nc.dram_tensor("protosT", [128, CP], BF16,
                             kind="ExternalInput").ap()
    out_sumexp = nc.dram_tensor("sumexp", [128, nt], F32,
                                kind="ExternalOutput").ap()
    out_picked = nc.dram_tensor("picked", [128, nt], F32,
                                kind="ExternalOutput").ap()

    NEWT_COLS = NEWT_CH * CH_T

    with tile.TileContext(nc) as tc, ExitStack() as ctx:
        const = ctx.enter_context(tc.tile_pool(name="const", bufs=1))
        frpool = ctx.enter_context(tc.tile_pool(name="fr", bufs=NEWT_CH + 3))
        f2pool = ctx.enter_context(tc.tile_pool(name="f2", bufs=3))
        fhpool = ctx.enter_context(tc.tile_pool(name="fh", bufs=3))
        ftpool = ctx.enter_context(tc.tile_pool(name="ft", bufs=3))
        ebpool = ctx.enter_context(tc.tile_pool(name="eb", bufs=4))
        v1pool = ctx.enter_context(tc.tile_pool(name="v1", bufs=2))
        v2pool = ctx.enter_context(tc.tile_pool(name="v2", bufs=2))
        v3pool = ctx.enter_context(tc.tile_pool(name="v3", bufs=2))
        u1pool = ctx.enter_context(tc.tile_pool(name="u1", bufs=2))
        u2pool = ctx.enter_context(tc.tile_pool(name="u2", bufs=2))
        u3pool = ctx.enter_context(tc.tile_pool(name="u3", bufs=2))
        ntpool = ctx.enter_context(tc.tile_pool(name="nt", bufs=2))
        lpool = ctx.enter_context(tc.tile_pool(name="lp", bufs=2,
                                               space="PSUM"))
        ppool = ctx.enter_context(tc.tile_pool(name="pp", bufs=1,
                                               space="PSUM"))

        protosT_sb = const.tile([128, CP], BF16)
        nc.sync.dma_start(protosT_sb[:], protosT[:, :])
        gT2_sb = const.tile([128, nt], BF16)
        nc.sync.dma_start(gT2_sb[:], gT2[:, :])
        kmagic = const.tile([128, NEWT_COLS], I32)
        nc.vector.memset(kmagic[:], 0x5F3759DF)

        q_buf = const.tile([128, nt], F32)
        sT_buf = const.tile([128, nt], BF16)
        sum_buf = const.tile([128, nt], F32)

        picked_ps = [ppool.tile([128, half], F32, name=f"picked{i}")
                     for i in range(2)]

        def newton_rsqrt_scaled(dst_bf16, q_ap, w):
            """dst = (1/sqrt(q)) / TEMP elementwise on [128, w] f32 SBUF."""
            bs = ntpool.tile([128, NEWT_COLS], I32, tag="nt_bs")
            nc.vector.tensor_scalar(bs[:, 0:w], q_ap.bitcast(I32), 1,
                                    None, ALU.logical_shift_right)
            y0 = ntpool.tile([128, NEWT_COLS], I32, tag="nt_y0")
            nc.vector.tensor_tensor(y0[:, 0:w], kmagic[:, 0:w], bs[:, 0:w],
                                    ALU.subtract)
            y0f = y0[:, 0:w].bitcast(F32)
            t = ntpool.tile([128, NEWT_COLS], F32, tag="nt_t")
            y1 = ntpool.tile([128, NEWT_COLS], F32, tag="nt_y1")
            nc.vector.scalar_tensor_tensor(t[:, 0:w], q_ap, -0.5, y0f,
                                           ALU.mult, ALU.mult)
            nc.vector.tensor_tensor(t[:, 0:w], t[:, 0:w], y0f, ALU.mult)
            nc.vector.scalar_tensor_tensor(y1[:, 0:w], t[:, 0:w], 1.5, y0f,
                                           ALU.add, ALU.mult)
            nc.vector.scalar_tensor_tensor(t[:, 0:w], q_ap, -0.5,
                                           y1[:, 0:w], ALU.mult, ALU.mult)
            nc.vector.scalar_tensor_tensor(t[:, 0:w], t[:, 0:w], 1.0 / TEMP,
                                           y1[:, 0:w], ALU.mult, ALU.mult)
            nc.vector.scalar_tensor_tensor(dst_bf16, t[:, 0:w],
                                           1.5 / TEMP, y1[:, 0:w],
                                           ALU.add, ALU.mult)

        fr_tiles = {}
        n_batches = (nch + NEWT_CH - 1) // NEWT_CH

        for b in range(n_batches):
            cs = list(range(b * NEWT_CH, min((b + 1) * NEWT_CH, nch)))

            # phase A: load + square + q-tree per chunk
            for c in cs:
                fr = frpool.tile([128, CH_T, D], BF16)
                nc.sync.dma_start(fr[:], fR[c])
                fr_tiles[c] = fr
                f2 = f2pool.tile([128, CH_T, D], BF16)
                nc.scalar.activation(f2[:], fr[:],
                                     mybir.ActivationFunctionType.Square)
                v1 = v1pool.tile([128, CH_T, 64], BF16)
                nc.gpsimd.tensor_tensor(v1[:], f2[:, :, 0:64],
                                        f2[:, :, 64:128], ALU.add)
                v2 = v2pool.tile([128, CH_T, 32], BF16)
                nc.vector.tensor_tensor(v2[:], v1[:, :, 0:32],
                                        v1[:, :, 32:64], ALU.add)
                v3 = v3pool.tile([128, CH_T, 16], BF16)
                nc.vector.tensor_tensor(v3[:], v2[:, :, 0:16],
                                        v2[:, :, 16:32], ALU.add)
                nc.vector.tensor_reduce(
                    q_buf[:, c * CH_T:(c + 1) * CH_T], v3[:], AX.X, ALU.add)

            # newton for the batch
            lo = cs[0] * CH_T
            hi = (cs[-1] + 1) * CH_T
            newton_rsqrt_scaled(sT_buf[:, lo:hi], q_buf[:, lo:hi], hi - lo)

            # phase B: normalize + transpose + matmuls + exp + sumexp
            for ci, c in enumerate(cs):
                fr = fr_tiles.pop(c)
                t0 = c * CH_T
                fh = fhpool.tile([128, CH_T, D], BF16)
                sbc = sT_buf[:, t0:t0 + CH_T].to_broadcast((128, CH_T, D))
                if ci < 3:
                    nc.gpsimd.tensor_tensor(fh[:], fr[:], sbc, ALU.mult)
                else:
                    nc.vector.tensor_tensor(fh[:], fr[:], sbc, ALU.mult)
                ft = ftpool.tile([128, CH_T, D], BF16)
                nc.sync.dma_start_transpose(
                    ft[:], fh[:].rearrange("p t d -> p (t d)"))

                u1 = u1pool.tile([128, CH_T, 80], BF16)
                for h in range(2):  # two 9-tile exp batches per chunk
                    lp = lpool.tile([128, 3, 512], F32)
                    for j in range(EB):
                        jt = h * EB + j           # tile within chunk
                        t = t0 + jt               # global tile
                        slot = lp[:, j // 3, (j % 3) * SLOT:
                                  (j % 3) * SLOT + CP]
                        nc.tensor.matmul(slot, ft[:, jt, :],
                                         protosT_sb[:], start=True, stop=True)
                        ph, pc = (0, t) if t < half else (1, t - half)
                        nc.tensor.matmul(picked_ps[ph][:, pc:pc + 1],
                                         ft[:, jt, :], gT2_sb[:, t:t + 1],
                                         start=True, stop=True)
                    eb = ebpool.tile([128, EB, CP], BF16)
                    lp_4d = lp[:, :, 0:510].rearrange(
                        "p b (j x) -> p b j x", x=SLOT)[:, :, :, 0:CP]
                    nc.scalar.activation(
                        eb[:].rearrange("p (b j) x -> p b j x", j=3),
                        lp_4d, mybir.ActivationFunctionType.Exp,
                        bias=0.0, scale=1.0)
                    nc.vector.tensor_tensor(u1[:, h * EB:(h + 1) * EB, :],
                                            eb[:, :, 0:80],
                                            eb[:, :, 80:160], ALU.add)
                u2 = u2pool.tile([128, CH_T, 40], BF16)
                nc.vector.tensor_tensor(u2[:], u1[:, :, 0:40],
                                        u1[:, :, 40:80], ALU.add)
                u3 = u3pool.tile([128, CH_T, 20], BF16)
                nc.vector.tensor_tensor(u3[:], u2[:, :, 0:20],
                                        u2[:, :, 20:40], ALU.add)
                nc.vector.tensor_reduce(
                    sum_buf[:, t0:t0 + CH_T], u3[:], AX.X, ALU.add)

        nc.sync.dma_start(out_sumexp[:, :], sum_buf[:])
        picked_sb = const.tile([128, nt], F32)
        nc.vector.tensor_scalar(picked_sb[:, 0:half], picked_ps[0][:],
                                1.0, None, ALU.mult)
        nc.vector.tensor_scalar(picked_sb[:, half:nt],
                                picked_ps[1][:, 0:nt - half],
                                1.0, None, ALU.mult)
        nc.sync.dma_start(out_picked[:, :], picked_sb[:])

    nc.compile()
    return nc


def _get_nc(nt):
    if nt not in _NC_CACHE:
        _NC_CACHE[nt] = build_nc(nt)
    return _NC_CACHE[nt]


def _prep_inputs(features, labels, prototypes):
    """Sort rows by label, pad classes to tile boundaries, shard, block.

    Returns (in_maps, valid_masks [core][128, nt], nt)."""
    n = features.shape[0]
    labels = np.asarray(labels).astype(np.int64)
    order = np.argsort(labels, kind="stable")
    sorted_labels = labels[order]

    # class boundaries in sorted order
    counts = np.bincount(sorted_labels, minlength=C)
    tiles_per_class = (counts + 127) // 128          # [C]
    total_tiles = int(tiles_per_class.sum())

    # per-core tile count: multiple of CH_T covering total_tiles/8
    per = (total_tiles + N_CORES - 1) // N_CORES
    nt = ((per + CH_T - 1) // CH_T) * CH_T
    nt_total = nt * N_CORES
    assert nt_total >= total_tiles

    # row_idx[tile, j] = original row index or -1 (dummy)
    row_idx = np.full((nt_total, 128), -1, dtype=np.int64)
    tile_label = np.zeros(nt_total, dtype=np.int64)
    tpos = 0
    rpos = 0
    for ccls in range(C):
        cnt = int(counts[ccls])
        ntile = int(tiles_per_class[ccls])
        if ntile == 0:
            continue
        idx = order[rpos:rpos + cnt]
        rpos += cnt
        block = np.full(ntile * 128, -1, dtype=np.int64)
        block[:cnt] = idx
        row_idx[tpos:tpos + ntile] = block.reshape(ntile, 128)
        tile_label[tpos:tpos + ntile] = ccls
        tpos += ntile

    protos_bf = np.ascontiguousarray(prototypes).astype(np.float32)

    feats_bf = features.astype(ml_dtypes.bfloat16)
    dummy = np.zeros(D, dtype=ml_dtypes.bfloat16)
    dummy[0] = 1.0

    protosT_np = np.zeros((128, CP), dtype=ml_dtypes.bfloat16)
    protosT_np[:, 0:C] = protos_bf.T.astype(ml_dtypes.bfloat16)

    in_maps = []
    masks = []
    nch = nt // CH_T
    for core in range(N_CORES):
        ti = row_idx[core * nt:(core + 1) * nt]          # [nt, 128]
        tl = tile_label[core * nt:(core + 1) * nt]       # [nt]
        # gather rows -> [nt*128, D] bf16
        flat = ti.reshape(-1)
        fr = np.empty((nt * 128, D), dtype=ml_dtypes.bfloat16)
        valid = flat >= 0
        fr[valid] = feats_bf[flat[valid]]
        fr[~valid] = dummy
        # block: [nch, CH_T, 128, D] -> [nch, 128, CH_T, D]
        frb = np.ascontiguousarray(
            fr.reshape(nch, CH_T, 128, D).transpose(0, 2, 1, 3))
        gT2 = np.ascontiguousarray(
            protos_bf[tl].T).astype(ml_dtypes.bfloat16)   # [128, nt]
        in_maps.append({"fR": frb, "gT2": gT2, "protosT": protosT_np})
        masks.append(ti.T >= 0)                           # [128, nt]
    return in_maps, masks, nt


def _reduce_host(res, masks, n_total):
    total = 0.0
    for core in range(N_CORES):
        m = masks[core]
        if not m.any():
            continue
        sumexp = res.results[core]["sumexp"].astype(np.float64)
        picked = res.results[core]["picked"].astype(np.float64)
        se = sumexp[m] - 10.0    # remove the 10 zero-pad columns (exp(0)=1)
        total += (np.log(se) - picked[m]).sum()
    return np.float32(total / n_total)


def kernel(features, labels, prototypes):
    features = np.asarray(features, dtype=np.float32)
    labels = np.asarray(labels)
    prototypes = np.asarray(prototypes, dtype=np.float32)
    n = features.shape[0]
    in_maps, masks, nt = _prep_inputs(features, labels, prototypes)
    nc = _get_nc(nt)
    res = run_bass_kernel_spmd(nc, in_maps, core_ids=list(range(N_CORES)))
    return _reduce_host(res, masks, n)


if __name__ == "__main__":
    rng = np.random.default_rng(0)
    n = 40_000
    f = rng.normal(size=(n, D)).astype(np.float32)
    lab = rng.integers(0, C, size=n).astype(np.int64)
    p = rng.normal(size=(C, D)).astype(np.float32)
    p /= np.linalg.norm(p, axis=1, keepdims=True)
    got = kernel(f, lab, p)

    fh = f / np.maximum(np.linalg.norm(f, axis=1, keepdims=True), 1e-12)
    logits = fh @ p.T / TEMP
    m = logits.max(axis=1, keepdims=True)
    logz = np.log(np.exp(logits - m).sum(1)) + m[:, 0]
    picked = np.take_along_axis(logits, lab[:, None], axis=1)[:, 0]
    want = (logz - picked).mean()
    print("got:", got, "want:", want, "rel:", abs(got / want - 1))


# revision 18
# speedup vs baseline: 1.3235x; 1.3235x over previous
"""Trainium2 Bass kernel for ClassFeatureMemoryBank proto-contrastive loss (v7).

loss = mean_r [ logsumexp_c(f_hat_r . p_c / T) - (f_hat_r . p_label_r / T) ]

v7 strategy (vs v2 baseline): eliminate ALL per-tile ACT/DVE instructions.
  - Host: sort rows by label, pad each class to 128-row tiles -> every tile
    is single-label. Ship features ROW-major (fR), tile-blocked.
  - ACT (batched): f2 = Square(fR); expb = Exp(logits) with constant scale.
  - GPSIMD: first tree level of both reductions (sum over d; sum over c).
  - DVE (batched): rest of q-tree, Newton rsqrt (s=1/(T*||f||)) on [128,nt]
    layout, f_hat = fR * s (free-dim broadcast), sumexp tree.
  - DMA xbar: blocked transpose f_hatR -> f_hatT per 18-tile chunk.
  - PE: per tile ONE stationary f_hatT, two matmuls: logits (protosT,
    padded to 160 cols with zeros -> sumexp gets +10, host subtracts) and
    picked (gT2 = per-tile label prototype column, FD=1).
  - Host: loss = mean(log(sumexp-10) - picked) over valid rows.
"""
import sys

sys.path.insert(0, "/opt/trn_rl_repo")

import numpy as np
import ml_dtypes
from contextlib import ExitStack

import concourse.bass as bass
import concourse.tile as tile
from concourse import bacc, mybir
from concourse.bass_utils import run_bass_kernel_spmd

F32 = mybir.dt.float32
BF16 = mybir.dt.bfloat16
I32 = mybir.dt.int32
ALU = mybir.AluOpType
AX = mybir.AxisListType

N_CORES = 8
D = 128
C = 150
CP = 160              # padded class columns (150 real + 10 zero)
SLOT = 170            # psum slot stride (3 slots * 170 * 4B = 2040 <= bank)
TEMP = 0.15
CH_T = 18             # tiles per chunk
EB = 9                # tiles per exp batch (2 per chunk)
NEWT_CH = 8           # chunks per newton batch

N_FULL = 1_000_000

_NC_CACHE = {}


def build_nc(nt: int):
    """nt tiles of 128 rows per core; nt % CH_T == 0."""
    assert nt % CH_T == 0
    nch = nt // CH_T
    half = (nt + 1) // 2
    assert half <= 512

    nc = bacc.Bacc("TRN2", target_bir_lowering=False, debug=False)
    fR = nc.dram_tensor("fR", [nch, 128, CH_T, D], BF16,
                        kind="ExternalInput").ap()
    gT2 = nc.dram_tensor("gT2", [128, nt], BF16, kind="ExternalInput").ap()
    protosT = nc.dram_tensor("protosT", [128, CP], BF16,
                             kind="ExternalInput").ap()
    out_sumexp = nc.dram_tensor("sumexp", [128, nt], F32,
                                kind="ExternalOutput").ap()
    out_picked = nc.dram_tensor("picked", [128, nt], F32,
                                kind="ExternalOutput").ap()

    NEWT_COLS = NEWT_CH * CH_T

    with tile.TileContext(nc) as tc, ExitStack() as ctx:
        const = ctx.enter_context(tc.tile_pool(name="const", bufs=1))
        frpool = ctx.enter_context(tc.tile_pool(name="fr", bufs=NEWT_CH + 3))
        f2pool = ctx.enter_context(tc.tile_pool(name="f2", bufs=3))
        fhpool = ctx.enter_context(tc.tile_pool(name="fh", bufs=3))
        ftpool = ctx.enter_context(tc.tile_pool(name="ft", bufs=3))
        ebpool = ctx.enter_context(tc.tile_pool(name="eb", bufs=4))
        v1pool = ctx.enter_context(tc.tile_pool(name="v1", bufs=2))
        v2pool = ctx.enter_context(tc.tile_pool(name="v2", bufs=2))
        v3pool = ctx.enter_context(tc.tile_pool(name="v3", bufs=2))
        u1pool = ctx.enter_context(tc.tile_pool(name="u1", bufs=2))
        u2pool = ctx.enter_context(tc.tile_pool(name="u2", bufs=2))
        u3pool = ctx.enter_context(tc.tile_pool(name="u3", bufs=2))
        ntpool = ctx.enter_context(tc.tile_pool(name="nt", bufs=2))
        lpool = ctx.enter_context(tc.tile_pool(name="lp", bufs=2,
                                               space="PSUM"))
        ppool = ctx.enter_context(tc.tile_pool(name="pp", bufs=1,
                                               space="PSUM"))

        protosT_sb = const.tile([128, CP], BF16)
        nc.sync.dma_start(protosT_sb[:], protosT[:, :])
        gT2_sb = const.tile([128, nt], BF16)
        nc.sync.dma_start(gT2_sb[:], gT2[:, :])
        kmagic = const.tile([128, NEWT_COLS], I32)
        nc.vector.memset(kmagic[:], 0x5F3759DF)

        q_buf = const.tile([128, nt], F32)
        sT_buf = const.tile([128, nt], BF16)
        sum_buf = const.tile([128, nt], F32)

        picked_ps = [ppool.tile([128, half], F32, name=f"picked{i}")
                     for i in range(2)]

        def newton_rsqrt_scaled(dst_bf16, q_ap, w):
            """dst = (1/sqrt(q)) / TEMP elementwise on [128, w] f32 SBUF."""
            bs = ntpool.tile([128, NEWT_COLS], I32, tag="nt_bs")
            nc.vector.tensor_scalar(bs[:, 0:w], q_ap.bitcast(I32), 1,
                                    None, ALU.logical_shift_right)
            y0 = ntpool.tile([128, NEWT_COLS], I32, tag="nt_y0")
            nc.vector.tensor_tensor(y0[:, 0:w], kmagic[:, 0:w], bs[:, 0:w],
                                    ALU.subtract)
            y0f = y0[:, 0:w].bitcast(F32)
            t = ntpool.tile([128, NEWT_COLS], F32, tag="nt_t")
            y1 = ntpool.tile([128, NEWT_COLS], F32, tag="nt_y1")
            nc.vector.scalar_tensor_tensor(t[:, 0:w], q_ap, -0.5, y0f,
                                           ALU.mult, ALU.mult)
            nc.vector.tensor_tensor(t[:, 0:w], t[:, 0:w], y0f, ALU.mult)
            nc.vector.scalar_tensor_tensor(y1[:, 0:w], t[:, 0:w], 1.5, y0f,
                                           ALU.add, ALU.mult)
            nc.vector.scalar_tensor_tensor(t[:, 0:w], q_ap, -0.5,
                                           y1[:, 0:w], ALU.mult, ALU.mult)
            nc.vector.scalar_tensor_tensor(t[:, 0:w], t[:, 0:w], 1.0 / TEMP,
                                           y1[:, 0:w], ALU.mult, ALU.mult)
            nc.vector.scalar_tensor_tensor(dst_bf16, t[:, 0:w],
                                           1.5 / TEMP, y1[:, 0:w],
                                           ALU.add, ALU.mult)

        fr_tiles = {}
        n_batches = (nch + NEWT_CH - 1) // NEWT_CH

        for b in range(n_batches):
            cs = list(range(b * NEWT_CH, min((b + 1) * NEWT_CH, nch)))

            # phase A: load + square + q-tree per chunk
            for c in cs:
                fr = frpool.tile([128, CH_T, D], BF16)
                nc.sync.dma_start(fr[:], fR[c])
                fr_tiles[c] = fr
                f2 = f2pool.tile([128, CH_T, D], BF16)
                nc.scalar.activation(f2[:], fr[:],
                                     mybir.ActivationFunctionType.Square)
                v1 = v1pool.tile([128, CH_T, 64], BF16)
                nc.gpsimd.tensor_tensor(v1[:], f2[:, :, 0:64],
                                        f2[:, :, 64:128], ALU.add)
                v2 = v2pool.tile([128, CH_T, 32], BF16)
                nc.vector.tensor_tensor(v2[:], v1[:, :, 0:32],
                                        v1[:, :, 32:64], ALU.add)
                v3 = v3pool.tile([128, CH_T, 16], BF16)
                nc.vector.tensor_tensor(v3[:], v2[:, :, 0:16],
                                        v2[:, :, 16:32], ALU.add)
                nc.vector.tensor_reduce(
                    q_buf[:, c * CH_T:(c + 1) * CH_T], v3[:], AX.X, ALU.add)

            # newton for the batch
            lo = cs[0] * CH_T
            hi = (cs[-1] + 1) * CH_T
            newton_rsqrt_scaled(sT_buf[:, lo:hi], q_buf[:, lo:hi], hi - lo)

            # phase B: normalize + transpose + matmuls + exp + sumexp
            for ci, c in enumerate(cs):
                fr = fr_tiles.pop(c)
                t0 = c * CH_T
                fh = fhpool.tile([128, CH_T, D], BF16)
                sbc = sT_buf[:, t0:t0 + CH_T].to_broadcast((128, CH_T, D))
                if ci < 6:
                    nc.gpsimd.tensor_tensor(fh[:], fr[:], sbc, ALU.mult)
                else:
                    nc.vector.tensor_tensor(fh[:], fr[:], sbc, ALU.mult)
                ft = ftpool.tile([128, CH_T, D], BF16)
                nc.sync.dma_start_transpose(
                    ft[:], fh[:].rearrange("p t d -> p (t d)"))

                u1 = u1pool.tile([128, CH_T, 80], BF16)
                for h in range(2):  # two 9-tile exp batches per chunk
                    lp = lpool.tile([128, 3, 512], F32)
                    for j in range(EB):
                        jt = h * EB + j           # tile within chunk
                        t = t0 + jt               # global tile
                        slot = lp[:, j // 3, (j % 3) * SLOT:
                                  (j % 3) * SLOT + CP]
                        nc.tensor.matmul(slot, ft[:, jt, :],
                                         protosT_sb[:], start=True, stop=True)
                        ph, pc = (0, t) if t < half else (1, t - half)
                        nc.tensor.matmul(picked_ps[ph][:, pc:pc + 1],
                                         ft[:, jt, :], gT2_sb[:, t:t + 1],
                                         start=True, stop=True)
                    eb = ebpool.tile([128, EB, CP], BF16)
                    lp_4d = lp[:, :, 0:510].rearrange(
                        "p b (j x) -> p b j x", x=SLOT)[:, :, :, 0:CP]
                    nc.scalar.activation(
                        eb[:].rearrange("p (b j) x -> p b j x", j=3),
                        lp_4d, mybir.ActivationFunctionType.Exp,
                        bias=0.0, scale=1.0)
                    nc.vector.tensor_tensor(u1[:, h * EB:(h + 1) * EB, :],
                                            eb[:, :, 0:80],
                                            eb[:, :, 80:160], ALU.add)
                u2 = u2pool.tile([128, CH_T, 40], BF16)
                nc.vector.tensor_tensor(u2[:], u1[:, :, 0:40],
                                        u1[:, :, 40:80], ALU.add)
                u3 = u3pool.tile([128, CH_T, 20], BF16)
                nc.vector.tensor_tensor(u3[:], u2[:, :, 0:20],
                                        u2[:, :, 20:40], ALU.add)
                nc.vector.tensor_reduce(
                    sum_buf[:, t0:t0 + CH_T], u3[:], AX.X, ALU.add)

        nc.sync.dma_start(out_sumexp[:, :], sum_buf[:])
        picked_sb = const.tile([128, nt], F32)
        nc.vector.tensor_scalar(picked_sb[:, 0:half], picked_ps[0][:],
                                1.0, None, ALU.mult)
        nc.vector.tensor_scalar(picked_sb[:, half:nt],
                                picked_ps[1][:, 0:nt - half],
                                1.0, None, ALU.mult)
        nc.sync.dma_start(out_picked[:, :], picked_sb[:])

    nc.compile()
    return nc


def _get_nc(nt):
    if nt not in _NC_CACHE:
        _NC_CACHE[nt] = build_nc(nt)
    return _NC_CACHE[nt]


def _prep_inputs(features, labels, prototypes):
    """Sort rows by label, pad classes to tile boundaries, shard, block.

    Returns (in_maps, valid_masks [core][128, nt], nt)."""
    n = features.shape[0]
    labels = np.asarray(labels).astype(np.int64)
    order = np.argsort(labels, kind="stable")
    sorted_labels = labels[order]

    # class boundaries in sorted order
    counts = np.bincount(sorted_labels, minlength=C)
    tiles_per_class = (counts + 127) // 128          # [C]
    total_tiles = int(tiles_per_class.sum())

    # per-core tile count: multiple of CH_T covering total_tiles/8
    per = (total_tiles + N_CORES - 1) // N_CORES
    nt = ((per + CH_T - 1) // CH_T) * CH_T
    nt_total = nt * N_CORES
    assert nt_total >= total_tiles

    # row_idx[tile, j] = original row index or -1 (dummy)
    row_idx = np.full((nt_total, 128), -1, dtype=np.int64)
    tile_label = np.zeros(nt_total, dtype=np.int64)
    tpos = 0
    rpos = 0
    for ccls in range(C):
        cnt = int(counts[ccls])
        ntile = int(tiles_per_class[ccls])
        if ntile == 0:
            continue
        idx = order[rpos:rpos + cnt]
        rpos += cnt
        block = np.full(ntile * 128, -1, dtype=np.int64)
        block[:cnt] = idx
        row_idx[tpos:tpos + ntile] = block.reshape(ntile, 128)
        tile_label[tpos:tpos + ntile] = ccls
        tpos += ntile

    protos_bf = np.ascontiguousarray(prototypes).astype(np.float32)

    feats_bf = features.astype(ml_dtypes.bfloat16)
    dummy = np.zeros(D, dtype=ml_dtypes.bfloat16)
    dummy[0] = 1.0

    protosT_np = np.zeros((128, CP), dtype=ml_dtypes.bfloat16)
    protosT_np[:, 0:C] = protos_bf.T.astype(ml_dtypes.bfloat16)

    in_maps = []
    masks = []
    nch = nt // CH_T
    for core in range(N_CORES):
        ti = row_idx[core * nt:(core + 1) * nt]          # [nt, 128]
        tl = tile_label[core * nt:(core + 1) * nt]       # [nt]
        # gather rows -> [nt*128, D] bf16
        flat = ti.reshape(-1)
        fr = np.empty((nt * 128, D), dtype=ml_dtypes.bfloat16)
        valid = flat >= 0
        fr[valid] = feats_bf[flat[valid]]
        fr[~valid] = dummy
        # block: [nch, CH_T, 128, D] -> [nch, 128, CH_T, D]
        frb = np.ascontiguousarray(
            fr.reshape(nch, CH_T, 128, D).transpose(0, 2, 1, 3))
        gT2 = np.ascontiguousarray(
            protos_bf[tl].T).astype(ml_dtypes.bfloat16)   # [128, nt]
        in_maps.append({"fR": frb, "gT2": gT2, "protosT": protosT_np})
        masks.append(ti.T >= 0)                           # [128, nt]
    return in_maps, masks, nt


def _reduce_host(res, masks, n_total):
    total = 0.0
    for core in range(N_CORES):
        m = masks[core]
        if not m.any():
            continue
        sumexp = res.results[core]["sumexp"].astype(np.float64)
        picked = res.results[core]["picked"].astype(np.float64)
        se = sumexp[m] - 10.0    # remove the 10 zero-pad columns (exp(0)=1)
        total += (np.log(se) - picked[m]).sum()
    return np.float32(total / n_total)


def kernel(features, labels, prototypes):
    features = np.asarray(features, dtype=np.float32)
    labels = np.asarray(labels)
    prototypes = np.asarray(prototypes, dtype=np.float32)
    n = features.shape[0]
    in_maps, masks, nt = _prep_inputs(features, labels, prototypes)
    nc = _get_nc(nt)
    res = run_bass_kernel_spmd(nc, in_maps, core_ids=list(range(N_CORES)))
    return _reduce_host(res, masks, n)


if __name__ == "__main__":
    rng = np.random.default_rng(0)
    n = 40_000
    f = rng.normal(size=(n, D)).astype(np.float32)
    lab = rng.integers(0, C, size=n).astype(np.int64)
    p = rng.normal(size=(C, D)).astype(np.float32)
    p /= np.linalg.norm(p, axis=1, keepdims=True)
    got = kernel(f, lab, p)

    fh = f / np.maximum(np.linalg.norm(f, axis=1, keepdims=True), 1e-12)
    logits = fh @ p.T / TEMP
    m = logits.max(axis=1, keepdims=True)
    logz = np.log(np.exp(logits - m).sum(1)) + m[:, 0]
    picked = np.take_along_axis(logits, lab[:, None], axis=1)[:, 0]
    want = (logz - picked).mean()
    print("got:", got, "want:", want, "rel:", abs(got / want - 1))


# revision 19
# speedup vs baseline: 1.3943x; 1.0535x over previous
"""Trainium2 Bass kernel for ClassFeatureMemoryBank proto-contrastive loss (v7).

loss = mean_r [ logsumexp_c(f_hat_r . p_c / T) - (f_hat_r . p_label_r / T) ]

v7 strategy (vs v2 baseline): eliminate ALL per-tile ACT/DVE instructions.
  - Host: sort rows by label, pad each class to 128-row tiles -> every tile
    is single-label. Ship features ROW-major (fR), tile-blocked.
  - ACT (batched): f2 = Square(fR); expb = Exp(logits) with constant scale.
  - GPSIMD: first tree level of both reductions (sum over d; sum over c).
  - DVE (batched): rest of q-tree, Newton rsqrt (s=1/(T*||f||)) on [128,nt]
    layout, f_hat = fR * s (free-dim broadcast), sumexp tree.
  - DMA xbar: blocked transpose f_hatR -> f_hatT per 18-tile chunk.
  - PE: per tile ONE stationary f_hatT, two matmuls: logits (protosT,
    padded to 160 cols with zeros -> sumexp gets +10, host subtracts) and
    picked (gT2 = per-tile label prototype column, FD=1).
  - Host: loss = mean(log(sumexp-10) - picked) over valid rows.
"""
import sys

sys.path.insert(0, "/opt/trn_rl_repo")

import numpy as np
import ml_dtypes
from contextlib import ExitStack

import concourse.bass as bass
import concourse.tile as tile
from concourse import bacc, mybir
from concourse.bass_utils import run_bass_kernel_spmd

F32 = mybir.dt.float32
BF16 = mybir.dt.bfloat16
I32 = mybir.dt.int32
ALU = mybir.AluOpType
AX = mybir.AxisListType

N_CORES = 8
D = 128
C = 150
CP = 160              # padded class columns (150 real + 10 zero)
SLOT = 170            # psum slot stride (3 slots * 170 * 4B = 2040 <= bank)
TEMP = 0.15
CH_T = 18             # tiles per chunk
EB = 9                # tiles per exp batch (2 per chunk)
NEWT_CH = 8           # chunks per newton batch

N_FULL = 1_000_000

_NC_CACHE = {}


def build_nc(nt: int):
    """nt tiles of 128 rows per core; nt % CH_T == 0."""
    assert nt % CH_T == 0
    nch = nt // CH_T
    half = (nt + 1) // 2
    assert half <= 512

    nc = bacc.Bacc("TRN2", target_bir_lowering=False, debug=False)
    fR = nc.dram_tensor("fR", [nch, 128, CH_T, D], BF16,
                        kind="ExternalInput").ap()
    gT2 = nc.dram_tensor("gT2", [128, nt], BF16, kind="ExternalInput").ap()
    protosT = nc.dram_tensor("protosT", [128, CP], BF16,
                             kind="ExternalInput").ap()
    out_sumexp = nc.dram_tensor("sumexp", [128, nt], F32,
                                kind="ExternalOutput").ap()
    out_picked = nc.dram_tensor("picked", [128, nt], F32,
                                kind="ExternalOutput").ap()

    NEWT_COLS = NEWT_CH * CH_T

    with tile.TileContext(nc) as tc, ExitStack() as ctx:
        const = ctx.enter_context(tc.tile_pool(name="const", bufs=1))
        frpool = ctx.enter_context(tc.tile_pool(name="fr", bufs=NEWT_CH + 3))
        f2pool = ctx.enter_context(tc.tile_pool(name="f2", bufs=3))
        fhpool = ctx.enter_context(tc.tile_pool(name="fh", bufs=3))
        ftpool = ctx.enter_context(tc.tile_pool(name="ft", bufs=3))
        ebpool = ctx.enter_context(tc.tile_pool(name="eb", bufs=4))
        v1pool = ctx.enter_context(tc.tile_pool(name="v1", bufs=2))
        v2pool = ctx.enter_context(tc.tile_pool(name="v2", bufs=2))
        v3pool = ctx.enter_context(tc.tile_pool(name="v3", bufs=2))
        u1pool = ctx.enter_context(tc.tile_pool(name="u1", bufs=2))
        u2pool = ctx.enter_context(tc.tile_pool(name="u2", bufs=2))
        u3pool = ctx.enter_context(tc.tile_pool(name="u3", bufs=2))
        ntpool = ctx.enter_context(tc.tile_pool(name="nt", bufs=2))
        lpool = ctx.enter_context(tc.tile_pool(name="lp", bufs=2,
                                               space="PSUM"))
        ppool = ctx.enter_context(tc.tile_pool(name="pp", bufs=1,
                                               space="PSUM"))

        protosT_sb = const.tile([128, CP], BF16)
        nc.sync.dma_start(protosT_sb[:], protosT[:, :])
        gT2_sb = const.tile([128, nt], BF16)
        nc.sync.dma_start(gT2_sb[:], gT2[:, :])
        kmagic = const.tile([128, NEWT_COLS], I32)
        nc.vector.memset(kmagic[:], 0x5F3759DF)

        q_buf = const.tile([128, nt], F32)
        sT_buf = const.tile([128, nt], BF16)
        sum_buf = const.tile([128, nt], F32)

        picked_ps = [ppool.tile([128, half], F32, name=f"picked{i}")
                     for i in range(2)]

        def newton_rsqrt_scaled(dst_bf16, q_ap, w):
            """dst = (1/sqrt(q)) / TEMP elementwise on [128, w] f32 SBUF."""
            bs = ntpool.tile([128, NEWT_COLS], I32, tag="nt_bs")
            nc.vector.tensor_scalar(bs[:, 0:w], q_ap.bitcast(I32), 1,
                                    None, ALU.logical_shift_right)
            y0 = ntpool.tile([128, NEWT_COLS], I32, tag="nt_y0")
            nc.vector.tensor_tensor(y0[:, 0:w], kmagic[:, 0:w], bs[:, 0:w],
                                    ALU.subtract)
            y0f = y0[:, 0:w].bitcast(F32)
            t = ntpool.tile([128, NEWT_COLS], F32, tag="nt_t")
            y1 = ntpool.tile([128, NEWT_COLS], F32, tag="nt_y1")
            nc.vector.scalar_tensor_tensor(t[:, 0:w], q_ap, -0.5, y0f,
                                           ALU.mult, ALU.mult)
            nc.vector.tensor_tensor(t[:, 0:w], t[:, 0:w], y0f, ALU.mult)
            nc.vector.scalar_tensor_tensor(y1[:, 0:w], t[:, 0:w], 1.5, y0f,
                                           ALU.add, ALU.mult)
            nc.vector.scalar_tensor_tensor(t[:, 0:w], q_ap, -0.5,
                                           y1[:, 0:w], ALU.mult, ALU.mult)
            nc.vector.scalar_tensor_tensor(t[:, 0:w], t[:, 0:w], 1.0 / TEMP,
                                           y1[:, 0:w], ALU.mult, ALU.mult)
            nc.vector.scalar_tensor_tensor(dst_bf16, t[:, 0:w],
                                           1.5 / TEMP, y1[:, 0:w],
                                           ALU.add, ALU.mult)

        fr_tiles = {}
        n_batches = (nch + NEWT_CH - 1) // NEWT_CH

        for b in range(n_batches):
            cs = list(range(b * NEWT_CH, min((b + 1) * NEWT_CH, nch)))

            # phase A: load + square + q-tree per chunk
            for c in cs:
                fr = frpool.tile([128, CH_T, D], BF16)
                nc.sync.dma_start(fr[:], fR[c])
                fr_tiles[c] = fr
                f2 = f2pool.tile([128, CH_T, D], BF16)
                nc.scalar.activation(f2[:], fr[:],
                                     mybir.ActivationFunctionType.Square)
                v1 = v1pool.tile([128, CH_T, 64], BF16)
                nc.gpsimd.tensor_tensor(v1[:], f2[:, :, 0:64],
                                        f2[:, :, 64:128], ALU.add)
                v2 = v2pool.tile([128, CH_T, 32], BF16)
                nc.vector.tensor_tensor(v2[:], v1[:, :, 0:32],
                                        v1[:, :, 32:64], ALU.add)
                v3 = v3pool.tile([128, CH_T, 16], BF16)
                nc.vector.tensor_tensor(v3[:], v2[:, :, 0:16],
                                        v2[:, :, 16:32], ALU.add)
                nc.vector.tensor_reduce(
                    q_buf[:, c * CH_T:(c + 1) * CH_T], v3[:], AX.X, ALU.add)

            # newton for the batch
            lo = cs[0] * CH_T
            hi = (cs[-1] + 1) * CH_T
            newton_rsqrt_scaled(sT_buf[:, lo:hi], q_buf[:, lo:hi], hi - lo)

            # phase B: normalize + transpose + matmuls + exp + sumexp
            for ci, c in enumerate(cs):
                fr = fr_tiles.pop(c)
                t0 = c * CH_T
                fh = fhpool.tile([128, CH_T, D], BF16)
                HB = CH_T // 2
                sb_lo = sT_buf[:, t0:t0 + HB].to_broadcast((128, HB, D))
                sb_hi = sT_buf[:, t0 + HB:t0 + CH_T].to_broadcast((128, HB, D))
                nc.vector.tensor_tensor(fh[:, 0:HB, :], fr[:, 0:HB, :],
                                        sb_lo, ALU.mult)
                nc.gpsimd.tensor_tensor(fh[:, HB:CH_T, :], fr[:, HB:CH_T, :],
                                        sb_hi, ALU.mult)
                ft = ftpool.tile([128, CH_T, D], BF16)
                nc.sync.dma_start_transpose(
                    ft[:], fh[:].rearrange("p t d -> p (t d)"))

                u1 = u1pool.tile([128, CH_T, 80], BF16)
                for h in range(2):  # two 9-tile exp batches per chunk
                    lp = lpool.tile([128, 3, 512], F32)
                    for j in range(EB):
                        jt = h * EB + j           # tile within chunk
                        t = t0 + jt               # global tile
                        slot = lp[:, j // 3, (j % 3) * SLOT:
                                  (j % 3) * SLOT + CP]
                        nc.tensor.matmul(slot, ft[:, jt, :],
                                         protosT_sb[:], start=True, stop=True)
                        ph, pc = (0, t) if t < half else (1, t - half)
                        nc.tensor.matmul(picked_ps[ph][:, pc:pc + 1],
                                         ft[:, jt, :], gT2_sb[:, t:t + 1],
                                         start=True, stop=True)
                    eb = ebpool.tile([128, EB, CP], BF16)
                    lp_4d = lp[:, :, 0:510].rearrange(
                        "p b (j x) -> p b j x", x=SLOT)[:, :, :, 0:CP]
                    nc.scalar.activation(
                        eb[:].rearrange("p (b j) x -> p b j x", j=3),
                        lp_4d, mybir.ActivationFunctionType.Exp,
                        bias=0.0, scale=1.0)
                    nc.vector.tensor_tensor(u1[:, h * EB:(h + 1) * EB, :],
                                            eb[:, :, 0:80],
                                            eb[:, :, 80:160], ALU.add)
                u2 = u2pool.tile([128, CH_T, 40], BF16)
                nc.vector.tensor_tensor(u2[:], u1[:, :, 0:40],
                                        u1[:, :, 40:80], ALU.add)
                u3 = u3pool.tile([128, CH_T, 20], BF16)
                nc.vector.tensor_tensor(u3[:], u2[:, :, 0:20],
                                        u2[:, :, 20:40], ALU.add)
                nc.vector.tensor_reduce(
                    sum_buf[:, t0:t0 + CH_T], u3[:], AX.X, ALU.add)

        nc.sync.dma_start(out_sumexp[:, :], sum_buf[:])
        picked_sb = const.tile([128, nt], F32)
        nc.vector.tensor_scalar(picked_sb[:, 0:half], picked_ps[0][:],
                                1.0, None, ALU.mult)
        nc.vector.tensor_scalar(picked_sb[:, half:nt],
                                picked_ps[1][:, 0:nt - half],
                                1.0, None, ALU.mult)
        nc.sync.dma_start(out_picked[:, :], picked_sb[:])

    nc.compile()
    return nc


def _get_nc(nt):
    if nt not in _NC_CACHE:
        _NC_CACHE[nt] = build_nc(nt)
    return _NC_CACHE[nt]


def _prep_inputs(features, labels, prototypes):
    """Sort rows by label, pad classes to tile boundaries, shard, block.

    Returns (in_maps, valid_masks [core][128, nt], nt)."""
    n = features.shape[0]
    labels = np.asarray(labels).astype(np.int64)
    order = np.argsort(labels, kind="stable")
    sorted_labels = labels[order]

    # class boundaries in sorted order
    counts = np.bincount(sorted_labels, minlength=C)
    tiles_per_class = (counts + 127) // 128          # [C]
    total_tiles = int(tiles_per_class.sum())

    # per-core tile count: multiple of CH_T covering total_tiles/8
    per = (total_tiles + N_CORES - 1) // N_CORES
    nt = ((per + CH_T - 1) // CH_T) * CH_T
    nt_total = nt * N_CORES
    assert nt_total >= total_tiles

    # row_idx[tile, j] = original row index or -1 (dummy)
    row_idx = np.full((nt_total, 128), -1, dtype=np.int64)
    tile_label = np.zeros(nt_total, dtype=np.int64)
    tpos = 0
    rpos = 0
    for ccls in range(C):
        cnt = int(counts[ccls])
        ntile = int(tiles_per_class[ccls])
        if ntile == 0:
            continue
        idx = order[rpos:rpos + cnt]
        rpos += cnt
        block = np.full(ntile * 128, -1, dtype=np.int64)
        block[:cnt] = idx
        row_idx[tpos:tpos + ntile] = block.reshape(ntile, 128)
        tile_label[tpos:tpos + ntile] = ccls
        tpos += ntile

    protos_bf = np.ascontiguousarray(prototypes).astype(np.float32)

    feats_bf = features.astype(ml_dtypes.bfloat16)
    dummy = np.zeros(D, dtype=ml_dtypes.bfloat16)
    dummy[0] = 1.0

    protosT_np = np.zeros((128, CP), dtype=ml_dtypes.bfloat16)
    protosT_np[:, 0:C] = protos_bf.T.astype(ml_dtypes.bfloat16)

    in_maps = []
    masks = []
    nch = nt // CH_T
    for core in range(N_CORES):
        ti = row_idx[core * nt:(core + 1) * nt]          # [nt, 128]
        tl = tile_label[core * nt:(core + 1) * nt]       # [nt]
        # gather rows -> [nt*128, D] bf16
        flat = ti.reshape(-1)
        fr = np.empty((nt * 128, D), dtype=ml_dtypes.bfloat16)
        valid = flat >= 0
        fr[valid] = feats_bf[flat[valid]]
        fr[~valid] = dummy
        # block: [nch, CH_T, 128, D] -> [nch, 128, CH_T, D]
        frb = np.ascontiguousarray(
            fr.reshape(nch, CH_T, 128, D).transpose(0, 2, 1, 3))
        gT2 = np.ascontiguousarray(
            protos_bf[tl].T).astype(ml_dtypes.bfloat16)   # [128, nt]
        in_maps.append({"fR": frb, "gT2": gT2, "protosT": protosT_np})
        masks.append(ti.T >= 0)                           # [128, nt]
    return in_maps, masks, nt


def _reduce_host(res, masks, n_total):
    total = 0.0
    for core in range(N_CORES):
        m = masks[core]
        if not m.any():
            continue
        sumexp = res.results[core]["sumexp"].astype(np.float64)
        picked = res.results[core]["picked"].astype(np.float64)
        se = sumexp[m] - 10.0    # remove the 10 zero-pad columns (exp(0)=1)
        total += (np.log(se) - picked[m]).sum()
    return np.float32(total / n_total)


def kernel(features, labels, prototypes):
    features = np.asarray(features, dtype=np.float32)
    labels = np.asarray(labels)
    prototypes = np.asarray(prototypes, dtype=np.float32)
    n = features.shape[0]
    in_maps, masks, nt = _prep_inputs(features, labels, prototypes)
    nc = _get_nc(nt)
    res = run_bass_kernel_spmd(nc, in_maps, core_ids=list(range(N_CORES)))
    return _reduce_host(res, masks, n)


if __name__ == "__main__":
    rng = np.random.default_rng(0)
    n = 40_000
    f = rng.normal(size=(n, D)).astype(np.float32)
    lab = rng.integers(0, C, size=n).astype(np.int64)
    p = rng.normal(size=(C, D)).astype(np.float32)
    p /= np.linalg.norm(p, axis=1, keepdims=True)
    got = kernel(f, lab, p)

    fh = f / np.maximum(np.linalg.norm(f, axis=1, keepdims=True), 1e-12)
    logits = fh @ p.T / TEMP
    m = logits.max(axis=1, keepdims=True)
    logz = np.log(np.exp(logits - m).sum(1)) + m[:, 0]
    picked = np.take_along_axis(logits, lab[:, None], axis=1)[:, 0]
    want = (logz - picked).mean()
    print("got:", got, "want:", want, "rel:", abs(got / want - 1))


# revision 20
# speedup vs baseline: 1.4632x; 1.0494x over previous
"""Trainium2 Bass kernel for ClassFeatureMemoryBank proto-contrastive loss (v7).

loss = mean_r [ logsumexp_c(f_hat_r . p_c / T) - (f_hat_r . p_label_r / T) ]

v7 strategy (vs v2 baseline): eliminate ALL per-tile ACT/DVE instructions.
  - Host: sort rows by label, pad each class to 128-row tiles -> every tile
    is single-label. Ship features ROW-major (fR), tile-blocked.
  - ACT (batched): f2 = Square(fR); expb = Exp(logits) with constant scale.
  - GPSIMD: first tree level of both reductions (sum over d; sum over c).
  - DVE (batched): rest of q-tree, Newton rsqrt (s=1/(T*||f||)) on [128,nt]
    layout, f_hat = fR * s (free-dim broadcast), sumexp tree.
  - DMA xbar: blocked transpose f_hatR -> f_hatT per 18-tile chunk.
  - PE: per tile ONE stationary f_hatT, two matmuls: logits (protosT,
    padded to 160 cols with zeros -> sumexp gets +10, host subtracts) and
    picked (gT2 = per-tile label prototype column, FD=1).
  - Host: loss = mean(log(sumexp-10) - picked) over valid rows.
"""
import sys

sys.path.insert(0, "/opt/trn_rl_repo")

import numpy as np
import ml_dtypes
from contextlib import ExitStack

import concourse.bass as bass
import concourse.tile as tile
from concourse import bacc, mybir
from concourse.bass_utils import run_bass_kernel_spmd

F32 = mybir.dt.float32
BF16 = mybir.dt.bfloat16
I32 = mybir.dt.int32
ALU = mybir.AluOpType
AX = mybir.AxisListType

N_CORES = 8
D = 128
C = 150
CP = 160              # padded class columns (150 real + 10 zero)
SLOT = 170            # psum slot stride (3 slots * 170 * 4B = 2040 <= bank)
TEMP = 0.15
CH_T = 18             # tiles per chunk
EB = 9                # tiles per exp batch (2 per chunk)
NEWT_CH = 8           # chunks per newton batch

N_FULL = 1_000_000

_NC_CACHE = {}


def build_nc(nt: int):
    """nt tiles of 128 rows per core; nt % CH_T == 0."""
    assert nt % CH_T == 0
    nch = nt // CH_T
    half = (nt + 1) // 2
    assert half <= 512

    nc = bacc.Bacc("TRN2", target_bir_lowering=False, debug=False)
    fR = nc.dram_tensor("fR", [nch, 128, CH_T, D], BF16,
                        kind="ExternalInput").ap()
    gT2 = nc.dram_tensor("gT2", [128, nt], BF16, kind="ExternalInput").ap()
    protosT = nc.dram_tensor("protosT", [128, CP], BF16,
                             kind="ExternalInput").ap()
    out_sumexp = nc.dram_tensor("sumexp", [128, nt], F32,
                                kind="ExternalOutput").ap()
    out_picked = nc.dram_tensor("picked", [128, nt], F32,
                                kind="ExternalOutput").ap()

    NEWT_COLS = NEWT_CH * CH_T

    with tile.TileContext(nc) as tc, ExitStack() as ctx:
        const = ctx.enter_context(tc.tile_pool(name="const", bufs=1))
        frpool = ctx.enter_context(tc.tile_pool(name="fr", bufs=NEWT_CH + 3))
        f2pool = ctx.enter_context(tc.tile_pool(name="f2", bufs=3))
        fhpool = ctx.enter_context(tc.tile_pool(name="fh", bufs=3))
        ftpool = ctx.enter_context(tc.tile_pool(name="ft", bufs=3))
        ebpool = ctx.enter_context(tc.tile_pool(name="eb", bufs=4))
        v1pool = ctx.enter_context(tc.tile_pool(name="v1", bufs=2))
        v2pool = ctx.enter_context(tc.tile_pool(name="v2", bufs=2))
        v3pool = ctx.enter_context(tc.tile_pool(name="v3", bufs=2))
        u1pool = ctx.enter_context(tc.tile_pool(name="u1", bufs=2))
        u2pool = ctx.enter_context(tc.tile_pool(name="u2", bufs=2))
        u3pool = ctx.enter_context(tc.tile_pool(name="u3", bufs=2))
        ntpool = ctx.enter_context(tc.tile_pool(name="nt", bufs=2))
        lpool = ctx.enter_context(tc.tile_pool(name="lp", bufs=2,
                                               space="PSUM"))
        ppool = ctx.enter_context(tc.tile_pool(name="pp", bufs=1,
                                               space="PSUM"))

        protosT_sb = const.tile([128, CP], BF16)
        nc.sync.dma_start(protosT_sb[:], protosT[:, :])
        gT2_sb = const.tile([128, nt], BF16)
        nc.sync.dma_start(gT2_sb[:], gT2[:, :])
        kmagic = const.tile([128, NEWT_COLS], I32)
        nc.vector.memset(kmagic[:], 0x5F3759DF)

        q_buf = const.tile([128, nt], F32)
        sT_buf = const.tile([128, nt], BF16)
        sum_buf = const.tile([128, nt], F32)

        picked_ps = [ppool.tile([128, half], F32, name=f"picked{i}")
                     for i in range(2)]

        def newton_rsqrt_scaled(dst_bf16, q_ap, w):
            """dst = (1/sqrt(q)) / TEMP elementwise on [128, w] f32 SBUF."""
            bs = ntpool.tile([128, NEWT_COLS], I32, tag="nt_bs")
            nc.vector.tensor_scalar(bs[:, 0:w], q_ap.bitcast(I32), 1,
                                    None, ALU.logical_shift_right)
            y0 = ntpool.tile([128, NEWT_COLS], I32, tag="nt_y0")
            nc.vector.tensor_tensor(y0[:, 0:w], kmagic[:, 0:w], bs[:, 0:w],
                                    ALU.subtract)
            y0f = y0[:, 0:w].bitcast(F32)
            t = ntpool.tile([128, NEWT_COLS], F32, tag="nt_t")
            y1 = ntpool.tile([128, NEWT_COLS], F32, tag="nt_y1")
            nc.vector.scalar_tensor_tensor(t[:, 0:w], q_ap, -0.5, y0f,
                                           ALU.mult, ALU.mult)
            nc.vector.tensor_tensor(t[:, 0:w], t[:, 0:w], y0f, ALU.mult)
            nc.vector.scalar_tensor_tensor(y1[:, 0:w], t[:, 0:w], 1.5, y0f,
                                           ALU.add, ALU.mult)
            nc.vector.scalar_tensor_tensor(t[:, 0:w], q_ap, -0.5,
                                           y1[:, 0:w], ALU.mult, ALU.mult)
            nc.vector.scalar_tensor_tensor(t[:, 0:w], t[:, 0:w], 1.0 / TEMP,
                                           y1[:, 0:w], ALU.mult, ALU.mult)
            nc.vector.scalar_tensor_tensor(dst_bf16, t[:, 0:w],
                                           1.5 / TEMP, y1[:, 0:w],
                                           ALU.add, ALU.mult)

        fr_tiles = {}
        n_batches = (nch + NEWT_CH - 1) // NEWT_CH

        for b in range(n_batches):
            cs = list(range(b * NEWT_CH, min((b + 1) * NEWT_CH, nch)))

            # phase A: load + square + q-tree per chunk
            for c in cs:
                fr = frpool.tile([128, CH_T, D], BF16)
                nc.sync.dma_start(fr[:], fR[c])
                fr_tiles[c] = fr
                f2 = f2pool.tile([128, CH_T, D], BF16)
                nc.scalar.activation(f2[:], fr[:],
                                     mybir.ActivationFunctionType.Square)
                v1 = v1pool.tile([128, CH_T, 64], BF16)
                HA = CH_T // 2
                nc.vector.tensor_tensor(v1[:, 0:HA, :], f2[:, 0:HA, 0:64],
                                        f2[:, 0:HA, 64:128], ALU.add)
                nc.gpsimd.tensor_tensor(v1[:, HA:CH_T, :], f2[:, HA:CH_T, 0:64],
                                        f2[:, HA:CH_T, 64:128], ALU.add)
                v2 = v2pool.tile([128, CH_T, 32], BF16)
                nc.vector.tensor_tensor(v2[:], v1[:, :, 0:32],
                                        v1[:, :, 32:64], ALU.add)
                v3 = v3pool.tile([128, CH_T, 16], BF16)
                nc.vector.tensor_tensor(v3[:], v2[:, :, 0:16],
                                        v2[:, :, 16:32], ALU.add)
                nc.vector.tensor_reduce(
                    q_buf[:, c * CH_T:(c + 1) * CH_T], v3[:], AX.X, ALU.add)

            # newton for the batch
            lo = cs[0] * CH_T
            hi = (cs[-1] + 1) * CH_T
            newton_rsqrt_scaled(sT_buf[:, lo:hi], q_buf[:, lo:hi], hi - lo)

            # phase B: normalize + transpose + matmuls + exp + sumexp
            for ci, c in enumerate(cs):
                fr = fr_tiles.pop(c)
                t0 = c * CH_T
                fh = fhpool.tile([128, CH_T, D], BF16)
                HB = CH_T // 2
                sb_lo = sT_buf[:, t0:t0 + HB].to_broadcast((128, HB, D))
                sb_hi = sT_buf[:, t0 + HB:t0 + CH_T].to_broadcast((128, HB, D))
                nc.vector.tensor_tensor(fh[:, 0:HB, :], fr[:, 0:HB, :],
                                        sb_lo, ALU.mult)
                nc.gpsimd.tensor_tensor(fh[:, HB:CH_T, :], fr[:, HB:CH_T, :],
                                        sb_hi, ALU.mult)
                ft = ftpool.tile([128, CH_T, D], BF16)
                nc.sync.dma_start_transpose(
                    ft[:], fh[:].rearrange("p t d -> p (t d)"))

                u1 = u1pool.tile([128, CH_T, 80], BF16)
                for h in range(2):  # two 9-tile exp batches per chunk
                    lp = lpool.tile([128, 3, 512], F32)
                    for j in range(EB):
                        jt = h * EB + j           # tile within chunk
                        t = t0 + jt               # global tile
                        slot = lp[:, j // 3, (j % 3) * SLOT:
                                  (j % 3) * SLOT + CP]
                        nc.tensor.matmul(slot, ft[:, jt, :],
                                         protosT_sb[:], start=True, stop=True)
                        ph, pc = (0, t) if t < half else (1, t - half)
                        nc.tensor.matmul(picked_ps[ph][:, pc:pc + 1],
                                         ft[:, jt, :], gT2_sb[:, t:t + 1],
                                         start=True, stop=True)
                    eb = ebpool.tile([128, EB, CP], BF16)
                    lp_4d = lp[:, :, 0:510].rearrange(
                        "p b (j x) -> p b j x", x=SLOT)[:, :, :, 0:CP]
                    nc.scalar.activation(
                        eb[:].rearrange("p (b j) x -> p b j x", j=3),
                        lp_4d, mybir.ActivationFunctionType.Exp,
                        bias=0.0, scale=1.0)
                    nc.vector.tensor_tensor(u1[:, h * EB:(h + 1) * EB, :],
                                            eb[:, :, 0:80],
                                            eb[:, :, 80:160], ALU.add)
                u2 = u2pool.tile([128, CH_T, 40], BF16)
                nc.vector.tensor_tensor(u2[:], u1[:, :, 0:40],
                                        u1[:, :, 40:80], ALU.add)
                u3 = u3pool.tile([128, CH_T, 20], BF16)
                nc.vector.tensor_tensor(u3[:], u2[:, :, 0:20],
                                        u2[:, :, 20:40], ALU.add)
                nc.vector.tensor_reduce(
                    sum_buf[:, t0:t0 + CH_T], u3[:], AX.X, ALU.add)

        nc.sync.dma_start(out_sumexp[:, :], sum_buf[:])
        picked_sb = const.tile([128, nt], F32)
        nc.vector.tensor_scalar(picked_sb[:, 0:half], picked_ps[0][:],
                                1.0, None, ALU.mult)
        nc.vector.tensor_scalar(picked_sb[:, half:nt],
                                picked_ps[1][:, 0:nt - half],
                                1.0, None, ALU.mult)
        nc.sync.dma_start(out_picked[:, :], picked_sb[:])

    nc.compile()
    return nc


def _get_nc(nt):
    if nt not in _NC_CACHE:
        _NC_CACHE[nt] = build_nc(nt)
    return _NC_CACHE[nt]


def _prep_inputs(features, labels, prototypes):
    """Sort rows by label, pad classes to tile boundaries, shard, block.

    Returns (in_maps, valid_masks [core][128, nt], nt)."""
    n = features.shape[0]
    labels = np.asarray(labels).astype(np.int64)
    order = np.argsort(labels, kind="stable")
    sorted_labels = labels[order]

    # class boundaries in sorted order
    counts = np.bincount(sorted_labels, minlength=C)
    tiles_per_class = (counts + 127) // 128          # [C]
    total_tiles = int(tiles_per_class.sum())

    # per-core tile count: multiple of CH_T covering total_tiles/8
    per = (total_tiles + N_CORES - 1) // N_CORES
    nt = ((per + CH_T - 1) // CH_T) * CH_T
    nt_total = nt * N_CORES
    assert nt_total >= total_tiles

    # row_idx[tile, j] = original row index or -1 (dummy)
    row_idx = np.full((nt_total, 128), -1, dtype=np.int64)
    tile_label = np.zeros(nt_total, dtype=np.int64)
    tpos = 0
    rpos = 0
    for ccls in range(C):
        cnt = int(counts[ccls])
        ntile = int(tiles_per_class[ccls])
        if ntile == 0:
            continue
        idx = order[rpos:rpos + cnt]
        rpos += cnt
        block = np.full(ntile * 128, -1, dtype=np.int64)
        block[:cnt] = idx
        row_idx[tpos:tpos + ntile] = block.reshape(ntile, 128)
        tile_label[tpos:tpos + ntile] = ccls
        tpos += ntile

    protos_bf = np.ascontiguousarray(prototypes).astype(np.float32)

    feats_bf = features.astype(ml_dtypes.bfloat16)
    dummy = np.zeros(D, dtype=ml_dtypes.bfloat16)
    dummy[0] = 1.0

    protosT_np = np.zeros((128, CP), dtype=ml_dtypes.bfloat16)
    protosT_np[:, 0:C] = protos_bf.T.astype(ml_dtypes.bfloat16)

    in_maps = []
    masks = []
    nch = nt // CH_T
    for core in range(N_CORES):
        ti = row_idx[core * nt:(core + 1) * nt]          # [nt, 128]
        tl = tile_label[core * nt:(core + 1) * nt]       # [nt]
        # gather rows -> [nt*128, D] bf16
        flat = ti.reshape(-1)
        fr = np.empty((nt * 128, D), dtype=ml_dtypes.bfloat16)
        valid = flat >= 0
        fr[valid] = feats_bf[flat[valid]]
        fr[~valid] = dummy
        # block: [nch, CH_T, 128, D] -> [nch, 128, CH_T, D]
        frb = np.ascontiguousarray(
            fr.reshape(nch, CH_T, 128, D).transpose(0, 2, 1, 3))
        gT2 = np.ascontiguousarray(
            protos_bf[tl].T).astype(ml_dtypes.bfloat16)   # [128, nt]
        in_maps.append({"fR": frb, "gT2": gT2, "protosT": protosT_np})
        masks.append(ti.T >= 0)                           # [128, nt]
    return in_maps, masks, nt


def _reduce_host(res, masks, n_total):
    total = 0.0
    for core in range(N_CORES):
        m = masks[core]
        if not m.any():
            continue
        sumexp = res.results[core]["sumexp"].astype(np.float64)
        picked = res.results[core]["picked"].astype(np.float64)
        se = sumexp[m] - 10.0    # remove the 10 zero-pad columns (exp(0)=1)
        total += (np.log(se) - picked[m]).sum()
    return np.float32(total / n_total)


def kernel(features, labels, prototypes):
    features = np.asarray(features, dtype=np.float32)
    labels = np.asarray(labels)
    prototypes = np.asarray(prototypes, dtype=np.float32)
    n = features.shape[0]
    in_maps, masks, nt = _prep_inputs(features, labels, prototypes)
    nc = _get_nc(nt)
    res = run_bass_kernel_spmd(nc, in_maps, core_ids=list(range(N_CORES)))
    return _reduce_host(res, masks, n)


if __name__ == "__main__":
    rng = np.random.default_rng(0)
    n = 40_000
    f = rng.normal(size=(n, D)).astype(np.float32)
    lab = rng.integers(0, C, size=n).astype(np.int64)
    p = rng.normal(size=(C, D)).astype(np.float32)
    p /= np.linalg.norm(p, axis=1, keepdims=True)
    got = kernel(f, lab, p)

    fh = f / np.maximum(np.linalg.norm(f, axis=1, keepdims=True), 1e-12)
    logits = fh @ p.T / TEMP
    m = logits.max(axis=1, keepdims=True)
    logz = np.log(np.exp(logits - m).sum(1)) + m[:, 0]
    picked = np.take_along_axis(logits, lab[:, None], axis=1)[:, 0]
    want = (logz - picked).mean()
    print("got:", got, "want:", want, "rel:", abs(got / want - 1))
